# revision 6
# baseline (speedup 1.0000x reference)
"""Causal self-attention Trainium2 kernel (8 NeuronCores, SPMD).

Problem (hardcoded): B=2, T=2048, C=1024, H=16 heads, d=64.
  qkv = x @ W_qkv ; per-head causal softmax attention ; out @ W_proj.

Sharding: core m (0..7) handles batch b = m//4 and head group g = m%4
(heads 4g..4g+3). Each core computes q/k/v for its 4 heads (256 of the
3072 W_qkv columns), full TxT causal attention for those heads, and a
partial projection y_m = att_m @ W_proj[256g:256g+256, :].  The host
sums the 4 partials per batch (row-split tensor parallel reduce).

Device kernel layout notes (scores kept TRANSPOSED: [key j, query i]):
  - x is fed pre-transposed per batch: xT [C, T] (fp16).
  - qkv^T computed as matmul(lhsT=W block, rhs=xT block): q^T/k^T land
    in [head-ch, T] layout, exactly what QK^T needs (contract over d).
  - v is computed in natural [T, ch] layout (lhsT=xT block, rhs=Wv) and
    stored with an extra ones half per head, so the AV matmul also
    yields the softmax denominators on the opposite 64 partitions.
  - Off-diagonal scores^T tiles [128 j, 512 i].  The 4 diagonal
    j-blocks of each 512-query chunk are computed per 256-query column
    (N=256 matmuls): QK skips the strictly-upper quarter, exp covers
    25% fewer elements; the unused quarter of the shared ew tile is
    zeroed so AV keeps v1's uniform N=512 accumulation groups.
  - Causal masks via gpsimd.affine_select on the 4 partially-masked
    [128,256] tiles per (pair, h2, chunk).
  - softmax divide: reciprocal_approx_fast (fp32, DVE) of the psum sum
    rows, then in-place multiply of the shifted data rows in attT.
  - Engine balance: exp stream on Scalar; k/v copies + tails on DVE;
    q copy + half the y copies on Scalar; masks + ew memset on GpSimd.
  - Attention pipelines both head pairs interleaved (1-group lookahead
    each); proj for chunk t-1 is emitted between qkv(t) and
    attention(t) so its attT inputs are long ready and y DMAs spread.
"""

import numpy as np

import concourse.bass as bass
import concourse.mybir as mybir
import concourse.tile as tile
from concourse import bacc

FP32 = mybir.dt.float32
FP16 = mybir.dt.float16
AF = mybir.ActivationFunctionType
ALU = mybir.AluOpType

B, T_FULL, C_FULL, H_FULL, D_HEAD = 2, 2048, 1024, 16, 64
N_CORES = 8


def build_nc(T=T_FULL, C=C_FULL, HD=4, D=D_HEAD, n_cores=N_CORES):
    """Build the per-core Bass program. HD = heads per core."""
    CD = HD * D              # device head channels (256)
    CB = C // 128            # contraction blocks over x/W channels
    ICH = 512                # query-chunk width
    S = ICH // 128           # j-blocks per query chunk on the diagonal
    TC = 512                 # token chunk in qkv phase
    NTC = T // TC
    NTB = T // 128           # 128-token blocks (= key blocks)
    NPAIR = HD // 2
    JQK = CD // 128          # q (and k) 128-wide column blocks
    assert JQK == NPAIR and T % ICH == 0 and C % 512 == 0
    softmax_scale = 1.0 / float(np.sqrt(D))

    nc = bacc.Bacc(
        "TRN2", target_bir_lowering=False, debug=False, num_devices=n_cores
    )
    xT = nc.dram_tensor("xT", [C, T], FP16, kind="ExternalInput").ap()
    wqkv = nc.dram_tensor("wqkv", [C, 3 * CD], FP16, kind="ExternalInput").ap()
    wp = nc.dram_tensor("wp", [CD, C], FP16, kind="ExternalInput").ap()
    y = nc.dram_tensor("y", [T, C], FP16, kind="ExternalOutput").ap()

    with tile.TileContext(nc) as tc:
        with (
            tc.tile_pool(name="consts", bufs=1) as consts,
            tc.tile_pool(name="xt", bufs=2 * CB) as xt_pool,
            tc.tile_pool(name="ew", bufs=8) as ew_pool,
            tc.tile_pool(name="ewd", bufs=6) as ewd_pool,
            tc.tile_pool(name="small", bufs=4) as small_pool,
            tc.tile_pool(name="ysb", bufs=4) as ysb_pool,
            tc.tile_pool(name="psb", bufs=2, space="PSUM") as ps_big,
            tc.tile_pool(name="psm", bufs=4, space="PSUM") as ps_med,
        ):
            # ---- resident tensors ----
            w_sb = consts.tile([128, CB, 3 * CD], FP16)
            wp_sb = consts.tile([128, CD // 128, C], FP16)
            qT = consts.tile([128, NPAIR, T], FP16)
            kT = consts.tile([128, NPAIR, T], FP16)
            # v with 64 replicated ones-columns per head: the AV matmul
            # (M=128) then emits data on one partition half and the
            # softmax denominator replicated across the other half --
            # denominator broadcast comes for free.  Even heads (h2=0):
            # [ones | v] -> av rows 0-63 = sums, 64-127 = data.  Odd
            # heads: [v | ones] -> av rows 0-63 = data, 64-127 = sums.
            vS = consts.tile([128, NTB, NPAIR, 2, 2, D], FP16)
            attT = consts.tile([128, NPAIR, T], FP16)

            # PE warmup probe: a matmul with no DMA dependencies, to
            # measure how early the PE can start at all.
            probe_src = consts.tile([128, 128], FP16)
            nc.gpsimd.memset(probe_src, 1.0)
            probe_ps = ps_med.tile([128, 512], FP32, tag="med")
            nc.tensor.matmul(
                probe_ps[:, 0:128], probe_src, probe_src,
                start=True, stop=True,
            )

            # ---- initial DMAs, in first-consumer order ----
            def issue_xt(t):
                xts = []
                for cb in range(CB):
                    xt_t = xt_pool.tile([128, TC], FP16, tag="xt")
                    nc.sync.dma_start(
                        xt_t, xT[128 * cb : 128 * (cb + 1), TC * t : TC * (t + 1)]
                    )
                    xts.append(xt_t)
                return xts

            xts_cur = issue_xt(0)
            w_r = wqkv.rearrange("(po pi) f -> pi po f", pi=128)
            for cb in range(CB):       # q+k columns, one piece per cb
                nc.sync.dma_start(
                    w_sb[:, cb, 0 : 2 * CD], w_r[:, cb, 0 : 2 * CD]
                )
            for cb in range(CB):       # v columns
                nc.sync.dma_start(
                    w_sb[:, cb, 2 * CD : 3 * CD], w_r[:, cb, 2 * CD : 3 * CD]
                )
            wp_r = wp.rearrange("(po pi) f -> pi po f", pi=128)
            for po in range(CD // 128):
                nc.sync.dma_start(wp_sb[:, po, :], wp_r[:, po, :])
            nc.vector.memset(vS[:, :, :, 0, 0, :], 1.0)
            nc.vector.memset(vS[:, :, :, 1, 1, :], 1.0)

            # ========== phase 1: qkv for one token chunk ==========
            def qkv_chunk(t, xts):
                # q^T / k^T: one [128, 2*TC] psum holds both column blocks
                for qk in range(2):       # 0 -> q, 1 -> k
                    p2 = ps_big.tile([128, JQK * TC], FP32, tag="big")
                    for jb in range(JQK):
                        co = CD * qk + 128 * jb
                        for cb in range(CB):
                            nc.tensor.matmul(
                                p2[:, TC * jb : TC * (jb + 1)],
                                w_sb[:, cb, co : co + 128],
                                xts[cb],
                                start=(cb == 0),
                                stop=(cb == CB - 1),
                            )
                    dst = qT if qk == 0 else kT
                    dst_ap = dst[:, :, TC * t : TC * (t + 1)]
                    src_ap = p2.rearrange("p (j f) -> p j f", j=JQK)
                    if qk == 0:
                        nc.scalar.copy(out=dst_ap, in_=src_ap)
                    else:
                        with nc.allow_low_precision("k copy fp16"):
                            nc.vector.tensor_copy(out=dst_ap, in_=src_ap)
                # v in natural layout, one 128-token block at a time
                for tb in range(TC // 128):
                    tb_g = t * (TC // 128) + tb
                    pvb = ps_big.tile([128, JQK * TC], FP32, tag="big")
                    pv = pvb[:, 0:CD]
                    for cb in range(CB):
                        nc.tensor.matmul(
                            pv,
                            xts[cb][:, 128 * tb : 128 * (tb + 1)],
                            w_sb[:, cb, 2 * CD : 3 * CD],
                            start=(cb == 0),
                            stop=(cb == CB - 1),
                        )
                    pv4 = pv.rearrange("p (pr h d) -> p pr h d", h=2, d=D)
                    with nc.allow_low_precision("v copy fp16"):
                        nc.vector.tensor_copy(
                            out=vS[:, tb_g, :, 0, 1, :], in_=pv4[:, :, 0, :]
                        )
                        nc.vector.tensor_copy(
                            out=vS[:, tb_g, :, 1, 0, :], in_=pv4[:, :, 1, :]
                        )

            # ================= phase 2: attention =================
            def emit_tail(p, ic, av):
                # Pure DVE+DMA tail, all DVE ops in-order on one engine:
                # stage the av psum tile to SBUF fp32 (frees the bank),
                # reciprocal_approx_fast on the full staged tile (only
                # the replicated-sums half is ever read; data lanes are
                # don't-care), cast to fp16, DMA partition-shift of the
                # data half into attT, in-place normalize.
                ics = slice(ICH * ic, ICH * (ic + 1))
                for h2 in range(2):
                    po = 64 * h2          # partition offset of the sums
                    s32 = small_pool.tile([128, ICH], FP32, tag="stage")
                    nc.vector.tensor_copy(out=s32, in_=av[h2])
                    r32 = small_pool.tile([128, ICH], FP32, tag="recip")
                    nc.vector.reciprocal_approx_fast(out=r32, in_=s32)
                    t16 = small_pool.tile([128, ICH], FP16, tag="tmp")
                    with nc.allow_low_precision("softmax raw fp16"):
                        nc.vector.tensor_copy(out=t16, in_=s32)
                    # data rows are on the opposite half from the sums
                    dst = attT[po : po + 64, p, ics]
                    nc.sync.dma_start(dst, t16[64 - po : 128 - po, :])
                    nc.vector.tensor_tensor(
                        dst, dst, r32[po : po + 64, :], ALU.mult
                    )

            def attention_chunk(t):
                n_off = S * t // 2        # off-diagonal groups (of 2 jb)
                avs = [
                    [
                        ps_med.tile(
                            [128, ICH], FP32, tag="med", name=f"av{p}{h2}"
                        )
                        for h2 in range(2)
                    ]
                    for p in range(NPAIR)
                ]
                # shared diagonal ew tiles, one per (p, h2): [128, 4, 512]
                # column j2, query-cols 0:256 from d0 / 256:512 from d1;
                # the unused quarter [2:4, 0:256] is zeroed.
                ewds = [[None, None] for _ in range(NPAIR)]

                def qk_off(p, g):
                    jbs = (2 * g, 2 * g + 1)
                    ews = []
                    for h2 in range(2):
                        po = 64 * h2
                        sc = ps_big.tile([128, 2 * ICH], FP32, tag="big")
                        for i_s, jb in enumerate(jbs):
                            nc.tensor.matmul(
                                sc[:, ICH * i_s : ICH * (i_s + 1)],
                                kT[po : po + 64, p, 128 * jb : 128 * (jb + 1)],
                                qT[po : po + 64, p, ICH * t : ICH * (t + 1)],
                                start=True,
                                stop=True,
                                tile_position=(po, 0),
                            )
                        ew = ew_pool.tile([128, 2, ICH], FP16, tag="ew")
                        nc.scalar.activation(
                            ew.rearrange("p a b -> p (a b)"),
                            sc,
                            AF.Exp,
                            scale=softmax_scale,
                        )
                        ews.append(ew)
                    return ("off", jbs, ews)

                def qk_diag(p, qc2):
                    """Diagonal, one 256-query column: key blocks
                    j2 = 0 .. 2*qc2+1 (the two topmost partially
                    masked)."""
                    off = S * t
                    nj = 2 * qc2 + 2
                    qs = slice(ICH * t + 256 * qc2, ICH * t + 256 * (qc2 + 1))
                    ecol = slice(256 * qc2, 256 * (qc2 + 1))
                    for h2 in range(2):
                        po = 64 * h2
                        if qc2 == 0:
                            ewd = ewd_pool.tile([128, S, ICH], FP16, tag="ewd")
                            nc.gpsimd.memset(ewd[:, 2:4, 0:256], 0.0)
                            ewds[p][h2] = ewd
                        else:
                            ewd = ewds[p][h2]
                        scb = ps_big.tile([128, 2 * ICH], FP32, tag="big")
                        scd = scb.rearrange("p (a b) -> p a b", b=256)
                        for j2 in range(nj):
                            nc.tensor.matmul(
                                scd[:, j2],
                                kT[
                                    po : po + 64,
                                    p,
                                    128 * (off + j2) : 128 * (off + j2 + 1),
                                ],
                                qT[po : po + 64, p, qs],
                                start=True,
                                stop=True,
                                tile_position=(po, 0),
                            )
                        nc.scalar.activation(
                            ewd[:, 0:nj, ecol],
                            scd[:, 0:nj],
                            AF.Exp,
                            scale=softmax_scale,
                        )
                        # the 2 topmost key blocks are partially masked:
                        # keep iff col - row >= 128*sbit
                        for sbit in range(2):
                            nc.gpsimd.affine_select(
                                out=ewd[:, 2 * qc2 + sbit, ecol],
                                in_=ewd[:, 2 * qc2 + sbit, ecol],
                                compare_op=ALU.is_ge,
                                fill=0.0,
                                base=-128 * sbit,
                                pattern=[[1, 256]],
                                channel_multiplier=-1,
                            )
                    return ("diag", qc2, None)

                def av_grp(p, grp):
                    kind, idx, ews = grp
                    J = S * (t + 1)
                    if kind == "off":
                        for h2 in range(2):
                            for i_s, jb in enumerate(idx):
                                nc.tensor.matmul(
                                    avs[p][h2],
                                    vS[:, jb, p, h2].rearrange(
                                        "p a b -> p (a b)"
                                    ),
                                    ews[h2][:, i_s],
                                    start=(jb == 0),
                                    stop=(jb == J - 1),
                                )
                    elif idx == 1:        # after d1: full diagonal AV
                        off = S * t
                        for h2 in range(2):
                            for j2 in range(S):
                                jb = off + j2
                                nc.tensor.matmul(
                                    avs[p][h2],
                                    vS[:, jb, p, h2].rearrange(
                                        "p a b -> p (a b)"
                                    ),
                                    ewds[p][h2][:, j2],
                                    start=(jb == 0),
                                    stop=(jb == J - 1),
                                )

                def make(p, s):
                    if s < n_off:
                        return qk_off(p, s)
                    return qk_diag(p, s - n_off)

                # merged pipeline across both pairs, 1-group lookahead:
                # QK(p, s) ... AV(p, s-1) keeps the PE fed while the
                # Scalar exp stream runs exactly one group behind.
                prev = [None] * NPAIR
                for s in range(n_off + 2):
                    for p in range(NPAIR):
                        cur = make(p, s)
                        if prev[p] is not None:
                            av_grp(p, prev[p])
                        prev[p] = cur
                for p in range(NPAIR):
                    av_grp(p, prev[p])
                    emit_tail(p, t, avs[p])

            # ================= phase 3: projection =================
            def proj_chunk(t):
                for tb in range(t * (TC // 128), (t + 1) * (TC // 128)):
                    for nck in range(C // 512):
                        py = ps_med.tile([128, 512], FP32, tag="med")
                        for p in range(NPAIR):
                            nc.tensor.matmul(
                                py,
                                attT[:, p, 128 * tb : 128 * (tb + 1)],
                                wp_sb[:, p, 512 * nck : 512 * (nck + 1)],
                                start=(p == 0),
                                stop=(p == NPAIR - 1),
                            )
                        y_sb = ysb_pool.tile([128, 512], FP16, tag="ysb")
                        if nck % 2 == 0:
                            nc.scalar.copy(out=y_sb, in_=py)
                        else:
                            with nc.allow_low_precision("y copy fp16"):
                                nc.vector.tensor_copy(out=y_sb, in_=py)
                        nc.sync.dma_start(
                            y[
                                128 * tb : 128 * (tb + 1),
                                512 * nck : 512 * (nck + 1),
                            ],
                            y_sb,
                        )

            for t in range(NTC):
                qkv_chunk(t, xts_cur)
                if t + 1 < NTC:
                    xts_cur = issue_xt(t + 1)
                if t > 0:
                    proj_chunk(t - 1)
                attention_chunk(t)
            proj_chunk(NTC - 1)
    nc.compile()
    return nc


def make_in_maps(x, W_qkv, W_proj, n_cores=N_CORES):
    """Host-side sharding: per-core fp16 inputs."""
    Bv, T, C = x.shape
    groups = n_cores // Bv
    CD = C // groups
    in_maps = []
    xT_b = [np.ascontiguousarray(x[b].T).astype(np.float16) for b in range(Bv)]
    for m in range(n_cores):
        b, g = m // groups, m % groups
        cols = slice(CD * g, CD * (g + 1))
        wqkv_dev = np.ascontiguousarray(
            np.concatenate(
                [W_qkv[:, 0:C][:, cols], W_qkv[:, C : 2 * C][:, cols],
                 W_qkv[:, 2 * C : 3 * C][:, cols]],
                axis=1,
            ).astype(np.float16)
        )
        wp_dev = np.ascontiguousarray(W_proj[cols, :]).astype(np.float16)
        in_maps.append({"xT": xT_b[b], "wqkv": wqkv_dev, "wp": wp_dev})
    return in_maps


_cache = {}


def _get_nc():
    if "nc" not in _cache:
        _cache["nc"] = build_nc()
    return _cache["nc"]


def run(x, W_qkv, W_proj, trace=False):
    """Run on hardware; returns (y_full, BassKernelResults)."""
    from concourse.bass_utils import run_bass_kernel_spmd

    nc = _get_nc()
    in_maps = make_in_maps(x, W_qkv, W_proj)
    res = run_bass_kernel_spmd(
        nc, in_maps, core_ids=list(range(N_CORES)), trace=trace
    )
    Bv, T, C = x.shape
    groups = N_CORES // Bv
    y_full = np.zeros((Bv, T, C), dtype=np.float32)
    for m in range(N_CORES):
        y_full[m // groups] += res.results[m]["y"].astype(np.float32)
    return y_full, res


def kernel(x, W_qkv, W_proj):
    y, _ = run(
        np.asarray(x, dtype=np.float32),
        np.asarray(W_qkv, dtype=np.float32),
        np.asarray(W_proj, dtype=np.float32),
    )
    return y


# revision 8
# speedup vs baseline: 1.1217x; 1.1217x over previous
"""Causal self-attention Trainium2 kernel (8 NeuronCores, SPMD).

Problem (hardcoded): B=2, T=2048, C=1024, H=16 heads, d=64.
  qkv = x @ W_qkv ; per-head causal softmax attention ; out @ W_proj.

Sharding: core m (0..7) handles batch b = m//4 and head group g = m%4
(heads 4g..4g+3). Each core computes q/k/v for its 4 heads (256 of the
3072 W_qkv columns), full TxT causal attention for those heads, and a
partial projection y_m = att_m @ W_proj[256g:256g+256, :].  The host
sums the 4 partials per batch (row-split tensor parallel reduce).

Device kernel layout notes (scores kept TRANSPOSED: [key j, query i]):
  - x is fed pre-transposed per batch: xT [C, T] (fp16).
  - qkv^T computed as matmul(lhsT=W block, rhs=xT block): q^T/k^T land
    in [head-ch, T] layout, exactly what QK^T needs (contract over d).
  - v is computed in natural [T, ch] layout (lhsT=xT block, rhs=Wv) and
    stored with an extra ones half per head, so the AV matmul also
    yields the softmax denominators on the opposite 64 partitions.
  - Off-diagonal scores^T tiles [128 j, 512 i].  The 4 diagonal
    j-blocks of each 512-query chunk are computed per 256-query column
    (N=256 matmuls): QK skips the strictly-upper quarter, exp covers
    25% fewer elements; the unused quarter of the shared ew tile is
    zeroed so AV keeps v1's uniform N=512 accumulation groups.
  - Causal masks via gpsimd.affine_select on the 4 partially-masked
    [128,256] tiles per (pair, h2, chunk).
  - softmax divide: reciprocal_approx_fast (fp32, DVE) of the psum sum
    rows, then in-place multiply of the shifted data rows in attT.
  - Engine balance: exp stream on Scalar; k/v copies + tails on DVE;
    q copy + half the y copies on Scalar; masks + ew memset on GpSimd.
  - Attention pipelines both head pairs interleaved (1-group lookahead
    each); proj for chunk t-1 is emitted between qkv(t) and
    attention(t) so its attT inputs are long ready and y DMAs spread.
"""

import numpy as np

import concourse.bass as bass
import concourse.mybir as mybir
import concourse.tile as tile
from concourse import bacc

FP32 = mybir.dt.float32
FP16 = mybir.dt.float16
AF = mybir.ActivationFunctionType
ALU = mybir.AluOpType

B, T_FULL, C_FULL, H_FULL, D_HEAD = 2, 2048, 1024, 16, 64
N_CORES = 8


def build_nc(T=T_FULL, C=C_FULL, HD=4, D=D_HEAD, n_cores=N_CORES):
    """Build the per-core Bass program. HD = heads per core."""
    CD = HD * D              # device head channels (256)
    CB = C // 128            # contraction blocks over x/W channels
    ICH = 512                # query-chunk width
    S = ICH // 128           # j-blocks per query chunk on the diagonal
    TC = 512                 # token chunk in qkv phase
    NTC = T // TC
    NTB = T // 128           # 128-token blocks (= key blocks)
    NPAIR = HD // 2
    JQK = CD // 128          # q (and k) 128-wide column blocks
    assert JQK == NPAIR and T % ICH == 0 and C % 512 == 0
    softmax_scale = 1.0 / float(np.sqrt(D))

    nc = bacc.Bacc(
        "TRN2", target_bir_lowering=False, debug=False, num_devices=n_cores
    )
    xT = nc.dram_tensor("xT", [C, T], FP16, kind="ExternalInput").ap()
    wqkv = nc.dram_tensor("wqkv", [C, 3 * CD], FP16, kind="ExternalInput").ap()
    wp = nc.dram_tensor("wp", [CD, C], FP16, kind="ExternalInput").ap()
    y = nc.dram_tensor("y", [T, C], FP16, kind="ExternalOutput").ap()

    with tile.TileContext(nc) as tc:
        with (
            tc.tile_pool(name="consts", bufs=1) as consts,
            tc.tile_pool(name="xt", bufs=2 * CB) as xt_pool,
            tc.tile_pool(name="ew", bufs=8) as ew_pool,
            tc.tile_pool(name="ewd", bufs=6) as ewd_pool,
            tc.tile_pool(name="small", bufs=4) as small_pool,
            tc.tile_pool(name="ysb", bufs=4) as ysb_pool,
            tc.tile_pool(name="psb", bufs=2, space="PSUM") as ps_big,
            tc.tile_pool(name="psm", bufs=4, space="PSUM") as ps_med,
        ):
            # ---- resident tensors ----
            w_sb = consts.tile([128, CB, 3 * CD], FP16)
            wp_sb = consts.tile([128, CD // 128, C], FP16)
            qT = consts.tile([128, NPAIR, T], FP16)
            kT = consts.tile([128, NPAIR, T], FP16)
            # v with 64 replicated ones-columns per head: the AV matmul
            # (M=128) then emits data on one partition half and the
            # softmax denominator replicated across the other half --
            # denominator broadcast comes for free.  Even heads (h2=0):
            # [ones | v] -> av rows 0-63 = sums, 64-127 = data.  Odd
            # heads: [v | ones] -> av rows 0-63 = data, 64-127 = sums.
            vS = consts.tile([128, NTB, NPAIR, 2, 2, D], FP16)
            attT = consts.tile([128, NPAIR, T], FP16)

            # PE warmup probe: a matmul with no DMA dependencies, to
            # measure how early the PE can start at all.
            probe_src = consts.tile([128, 128], FP16)
            nc.gpsimd.memset(probe_src, 1.0)
            probe_ps = ps_med.tile([128, 512], FP32, tag="med")
            nc.tensor.matmul(
                probe_ps[:, 0:128], probe_src, probe_src,
                start=True, stop=True,
            )

            # ---- initial DMAs, in first-consumer order ----
            def issue_xt(t):
                xts = []
                for cb in range(CB):
                    xt_t = xt_pool.tile([128, TC], FP16, tag="xt")
                    nc.sync.dma_start(
                        xt_t, xT[128 * cb : 128 * (cb + 1), TC * t : TC * (t + 1)]
                    )
                    xts.append(xt_t)
                return xts

            xts_cur = issue_xt(0)
            w_r = wqkv.rearrange("(po pi) f -> pi po f", pi=128)
            for cb in range(CB):       # q+k columns, one piece per cb
                nc.sync.dma_start(
                    w_sb[:, cb, 0 : 2 * CD], w_r[:, cb, 0 : 2 * CD]
                )
            for cb in range(CB):       # v columns
                nc.sync.dma_start(
                    w_sb[:, cb, 2 * CD : 3 * CD], w_r[:, cb, 2 * CD : 3 * CD]
                )
            wp_r = wp.rearrange("(po pi) f -> pi po f", pi=128)
            for po in range(CD // 128):
                nc.sync.dma_start(wp_sb[:, po, :], wp_r[:, po, :])
            nc.vector.memset(vS[:, :, :, 0, 0, :], 1.0)
            nc.vector.memset(vS[:, :, :, 1, 1, :], 1.0)

            # ========== phase 1: qkv for one token chunk ==========
            def qkv_chunk(t, xts):
                # q^T / k^T: one [128, 2*TC] psum holds both column blocks
                for qk in range(2):       # 0 -> q, 1 -> k
                    p2 = ps_big.tile([128, JQK * TC], FP32, tag="big")
                    for jb in range(JQK):
                        co = CD * qk + 128 * jb
                        for cb in range(CB):
                            nc.tensor.matmul(
                                p2[:, TC * jb : TC * (jb + 1)],
                                w_sb[:, cb, co : co + 128],
                                xts[cb],
                                start=(cb == 0),
                                stop=(cb == CB - 1),
                            )
                    dst = qT if qk == 0 else kT
                    dst_ap = dst[:, :, TC * t : TC * (t + 1)]
                    src_ap = p2.rearrange("p (j f) -> p j f", j=JQK)
                    if qk == 0:
                        nc.scalar.copy(out=dst_ap, in_=src_ap)
                    else:
                        with nc.allow_low_precision("k copy fp16"):
                            nc.vector.tensor_copy(out=dst_ap, in_=src_ap)
                # v in natural layout, one 128-token block at a time
                for tb in range(TC // 128):
                    tb_g = t * (TC // 128) + tb
                    pvb = ps_big.tile([128, JQK * TC], FP32, tag="big")
                    pv = pvb[:, 0:CD]
                    for cb in range(CB):
                        nc.tensor.matmul(
                            pv,
                            xts[cb][:, 128 * tb : 128 * (tb + 1)],
                            w_sb[:, cb, 2 * CD : 3 * CD],
                            start=(cb == 0),
                            stop=(cb == CB - 1),
                        )
                    pv4 = pv.rearrange("p (pr h d) -> p pr h d", h=2, d=D)
                    with nc.allow_low_precision("v copy fp16"):
                        nc.vector.tensor_copy(
                            out=vS[:, tb_g, :, 0, 1, :], in_=pv4[:, :, 0, :]
                        )
                        nc.vector.tensor_copy(
                            out=vS[:, tb_g, :, 1, 0, :], in_=pv4[:, :, 1, :]
                        )

            # ================= phase 2: attention =================
            def emit_tail(p, ic, av):
                # Pure DVE+DMA tail, all DVE ops in-order on one engine:
                # stage the av psum tile to SBUF fp32 (frees the bank),
                # reciprocal_approx_fast on the full staged tile (only
                # the replicated-sums half is ever read; data lanes are
                # don't-care), cast to fp16, DMA partition-shift of the
                # data half into attT, in-place normalize.
                ics = slice(ICH * ic, ICH * (ic + 1))
                for h2 in range(2):
                    po = 64 * h2          # partition offset of the sums
                    s32 = small_pool.tile([128, ICH], FP32, tag="stage")
                    nc.vector.tensor_copy(out=s32, in_=av[h2])
                    r32 = small_pool.tile([128, ICH], FP32, tag="recip")
                    nc.vector.reciprocal_approx_fast(out=r32, in_=s32)
                    t16 = small_pool.tile([128, ICH], FP16, tag="tmp")
                    with nc.allow_low_precision("softmax raw fp16"):
                        nc.vector.tensor_copy(out=t16, in_=s32)
                    # data rows are on the opposite half from the sums
                    dst = attT[po : po + 64, p, ics]
                    nc.sync.dma_start(dst, t16[64 - po : 128 - po, :])
                    nc.vector.tensor_tensor(
                        dst, dst, r32[po : po + 64, :], ALU.mult
                    )

            pending = None

            def attention_ic(p, t):
                nonlocal pending
                n_off = S * t // 2        # off-diagonal groups (of 2 jb)
                J = S * (t + 1)
                av = [
                    ps_med.tile([128, ICH], FP32, tag="med", name=f"av{h2}")
                    for h2 in range(2)
                ]
                # shared diagonal ew tiles, one per h2: [128, 4, 512] --
                # column j2, query-cols 0:256 from d0 / 256:512 from d1;
                # the unused quarter [2:4, 0:256] is zeroed.
                ewds = [None, None]

                def qk_off(g):
                    jbs = (2 * g, 2 * g + 1)
                    ews = []
                    for h2 in range(2):
                        po = 64 * h2
                        sc = ps_big.tile([128, 2 * ICH], FP32, tag="big")
                        for i_s, jb in enumerate(jbs):
                            nc.tensor.matmul(
                                sc[:, ICH * i_s : ICH * (i_s + 1)],
                                kT[po : po + 64, p, 128 * jb : 128 * (jb + 1)],
                                qT[po : po + 64, p, ICH * t : ICH * (t + 1)],
                                start=True,
                                stop=True,
                                tile_position=(po, 0),
                            )
                        ew = ew_pool.tile([128, 2, ICH], FP16, tag="ew")
                        nc.scalar.activation(
                            ew.rearrange("p a b -> p (a b)"),
                            sc,
                            AF.Exp,
                            scale=softmax_scale,
                        )
                        ews.append(ew)
                    return ("off", jbs, ews)

                def qk_diag(qc2):
                    """Diagonal, one 256-query column: key blocks
                    j2 = 0 .. 2*qc2+1 (the two topmost partially
                    masked)."""
                    off = S * t
                    nj = 2 * qc2 + 2
                    qs = slice(ICH * t + 256 * qc2, ICH * t + 256 * (qc2 + 1))
                    ecol = slice(256 * qc2, 256 * (qc2 + 1))
                    for h2 in range(2):
                        po = 64 * h2
                        if qc2 == 0:
                            ewd = ewd_pool.tile([128, S, ICH], FP16, tag="ewd")
                            nc.gpsimd.memset(ewd[:, 2:4, 0:256], 0.0)
                            ewds[h2] = ewd
                        else:
                            ewd = ewds[h2]
                        scb = ps_big.tile([128, 2 * ICH], FP32, tag="big")
                        scd = scb.rearrange("p (a b) -> p a b", b=256)
                        for j2 in range(nj):
                            nc.tensor.matmul(
                                scd[:, j2],
                                kT[
                                    po : po + 64,
                                    p,
                                    128 * (off + j2) : 128 * (off + j2 + 1),
                                ],
                                qT[po : po + 64, p, qs],
                                start=True,
                                stop=True,
                                tile_position=(po, 0),
                            )
                        nc.scalar.activation(
                            ewd[:, 0:nj, ecol],
                            scd[:, 0:nj],
                            AF.Exp,
                            scale=softmax_scale,
                        )
                        # the 2 topmost key blocks are partially masked:
                        # keep iff col - row >= 128*sbit
                        for sbit in range(2):
                            nc.gpsimd.affine_select(
                                out=ewd[:, 2 * qc2 + sbit, ecol],
                                in_=ewd[:, 2 * qc2 + sbit, ecol],
                                compare_op=ALU.is_ge,
                                fill=0.0,
                                base=-128 * sbit,
                                pattern=[[1, 256]],
                                channel_multiplier=-1,
                            )
                    return ("diag", qc2, None)

                def av_grp(grp):
                    kind, idx, ews = grp
                    if kind == "off":
                        for h2 in range(2):
                            for i_s, jb in enumerate(idx):
                                nc.tensor.matmul(
                                    av[h2],
                                    vS[:, jb, p, h2].rearrange(
                                        "p a b -> p (a b)"
                                    ),
                                    ews[h2][:, i_s],
                                    start=(jb == 0),
                                    stop=(jb == J - 1),
                                )
                    elif idx == 1:        # after d1: full diagonal AV
                        off = S * t
                        for h2 in range(2):
                            for j2 in range(S):
                                jb = off + j2
                                nc.tensor.matmul(
                                    av[h2],
                                    vS[:, jb, p, h2].rearrange(
                                        "p a b -> p (a b)"
                                    ),
                                    ewds[h2][:, j2],
                                    start=(jb == 0),
                                    stop=(jb == J - 1),
                                )

                def make(s):
                    if s < n_off:
                        return qk_off(s)
                    return qk_diag(s - n_off)

                # 1-group software pipeline as in the fast baseline:
                # QK(s+1) before AV(s); the previous (p, t)'s tail goes
                # after the first lookahead QK so the PE never waits on
                # the DVE tail chain.
                prev = make(0)
                for s in range(1, n_off + 2):
                    cur = make(s)
                    if pending is not None:
                        emit_tail(*pending)
                        pending = None
                    av_grp(prev)
                    prev = cur
                if pending is not None:
                    emit_tail(*pending)
                    pending = None
                av_grp(prev)
                pending = (p, t, av)

            # ================= phase 3: projection =================
            def proj_chunk(t):
                for tb in range(t * (TC // 128), (t + 1) * (TC // 128)):
                    for nck in range(C // 512):
                        py = ps_med.tile([128, 512], FP32, tag="med")
                        for p in range(NPAIR):
                            nc.tensor.matmul(
                                py,
                                attT[:, p, 128 * tb : 128 * (tb + 1)],
                                wp_sb[:, p, 512 * nck : 512 * (nck + 1)],
                                start=(p == 0),
                                stop=(p == NPAIR - 1),
                            )
                        y_sb = ysb_pool.tile([128, 512], FP16, tag="ysb")
                        if nck % 2 == 0:
                            nc.scalar.copy(out=y_sb, in_=py)
                        else:
                            with nc.allow_low_precision("y copy fp16"):
                                nc.vector.tensor_copy(out=y_sb, in_=py)
                        nc.sync.dma_start(
                            y[
                                128 * tb : 128 * (tb + 1),
                                512 * nck : 512 * (nck + 1),
                            ],
                            y_sb,
                        )

            for t in range(NTC):
                qkv_chunk(t, xts_cur)
                if t + 1 < NTC:
                    xts_cur = issue_xt(t + 1)
                if t > 0:
                    if pending is not None:
                        emit_tail(*pending)
                        pending = None
                    proj_chunk(t - 1)
                for p in range(NPAIR):
                    attention_ic(p, t)
            if pending is not None:
                emit_tail(*pending)
                pending = None
            proj_chunk(NTC - 1)
    nc.compile()
    return nc


def make_in_maps(x, W_qkv, W_proj, n_cores=N_CORES):
    """Host-side sharding: per-core fp16 inputs."""
    Bv, T, C = x.shape
    groups = n_cores // Bv
    CD = C // groups
    in_maps = []
    xT_b = [np.ascontiguousarray(x[b].T).astype(np.float16) for b in range(Bv)]
    for m in range(n_cores):
        b, g = m // groups, m % groups
        cols = slice(CD * g, CD * (g + 1))
        wqkv_dev = np.ascontiguousarray(
            np.concatenate(
                [W_qkv[:, 0:C][:, cols], W_qkv[:, C : 2 * C][:, cols],
                 W_qkv[:, 2 * C : 3 * C][:, cols]],
                axis=1,
            ).astype(np.float16)
        )
        wp_dev = np.ascontiguousarray(W_proj[cols, :]).astype(np.float16)
        in_maps.append({"xT": xT_b[b], "wqkv": wqkv_dev, "wp": wp_dev})
    return in_maps


_cache = {}


def _get_nc():
    if "nc" not in _cache:
        _cache["nc"] = build_nc()
    return _cache["nc"]


def run(x, W_qkv, W_proj, trace=False):
    """Run on hardware; returns (y_full, BassKernelResults)."""
    from concourse.bass_utils import run_bass_kernel_spmd

    nc = _get_nc()
    in_maps = make_in_maps(x, W_qkv, W_proj)
    res = run_bass_kernel_spmd(
        nc, in_maps, core_ids=list(range(N_CORES)), trace=trace
    )
    Bv, T, C = x.shape
    groups = N_CORES // Bv
    y_full = np.zeros((Bv, T, C), dtype=np.float32)
    for m in range(N_CORES):
        y_full[m // groups] += res.results[m]["y"].astype(np.float32)
    return y_full, res


def kernel(x, W_qkv, W_proj):
    y, _ = run(
        np.asarray(x, dtype=np.float32),
        np.asarray(W_qkv, dtype=np.float32),
        np.asarray(W_proj, dtype=np.float32),
    )
    return y


# revision 12
# speedup vs baseline: 1.1272x; 1.0049x over previous
"""Causal self-attention Trainium2 kernel (8 NeuronCores, SPMD).

Problem (hardcoded): B=2, T=2048, C=1024, H=16 heads, d=64.
  qkv = x @ W_qkv ; per-head causal softmax attention ; out @ W_proj.

Sharding: core m (0..7) handles batch b = m//4 and head group g = m%4
(heads 4g..4g+3). Each core computes q/k/v for its 4 heads (256 of the
3072 W_qkv columns), full TxT causal attention for those heads, and a
partial projection y_m = att_m @ W_proj[256g:256g+256, :].  The host
sums the 4 partials per batch (row-split tensor parallel reduce).

Device kernel layout notes (scores kept TRANSPOSED: [key j, query i]):
  - x is fed pre-transposed per batch: xT [C, T] (fp16).
  - Weights live in per-cb tiles (w_qk_t[cb] / w_v_t[cb]) so the first
    qkv matmul only waits for its own small DMA, not the whole weight
    load (tile-granular DMA dependencies).
  - qkv^T computed as matmul(lhsT=W block, rhs=xT block): q^T/k^T land
    in [head-ch, T] layout, exactly what QK^T needs (contract over d).
  - v is computed in natural [T, ch] layout and stored with an extra
    ones half per head, so the AV matmul also yields the softmax
    denominators on the opposite 64 partitions for free.
  - Off-diagonal scores^T tiles [128 j, 512 i].  The 4 diagonal
    j-blocks of each 512-query chunk are computed per 256-query column
    (N=256 matmuls, groups d0/d1): QK and exp skip the strictly-upper
    quarter.  AV is split left/right per 256-column half so every
    pipeline step has PE work (no zero-fill needed).
  - Causal masks via gpsimd.affine_select on the 4 partially-masked
    [128,256] tiles per (pair, h2, chunk).
  - softmax divide: stage av to SBUF fp32, reciprocal_approx_fast on
    the full staged tile (sums half replicated; data lanes dont-care),
    cast fp16, DMA partition-shift into attT, in-place normalize.
    All tail ops stay on the in-order DVE.
  - Global software pipeline: each unit (qkv section / attention pair)
    flushes the PREVIOUS unit's final AV group and softmax tail after
    its own first PE burst, so the PE never waits on the exp/mask
    chain at unit seams.
  - proj for chunk t-1 runs between qkv(t) and attention(t); the final
    chunk's projection accumulates the p=0 partials first so the PE
    covers the last tail chain, using 8 concurrent psum tiles.
"""

import numpy as np

import concourse.bass as bass
import concourse.mybir as mybir
import concourse.tile as tile
from concourse import bacc

FP32 = mybir.dt.float32
FP16 = mybir.dt.float16
AF = mybir.ActivationFunctionType
ALU = mybir.AluOpType

B, T_FULL, C_FULL, H_FULL, D_HEAD = 2, 2048, 1024, 16, 64
N_CORES = 8


def build_nc(T=T_FULL, C=C_FULL, HD=4, D=D_HEAD, n_cores=N_CORES):
    """Build the per-core Bass program. HD = heads per core."""
    CD = HD * D              # device head channels (256)
    CB = C // 128            # contraction blocks over x/W channels
    ICH = 512                # query-chunk width
    S = ICH // 128           # j-blocks per query chunk on the diagonal
    TC = 512                 # token chunk in qkv phase
    NTC = T // TC
    NTB = T // 128           # 128-token blocks (= key blocks)
    NPAIR = HD // 2
    JQK = CD // 128          # q (and k) 128-wide column blocks
    assert JQK == NPAIR and T % ICH == 0 and C % 512 == 0
    softmax_scale = 1.0 / float(np.sqrt(D))

    nc = bacc.Bacc(
        "TRN2", target_bir_lowering=False, debug=False, num_devices=n_cores
    )
    xT = nc.dram_tensor("xT", [C, T], FP16, kind="ExternalInput").ap()
    wqkv = nc.dram_tensor("wqkv", [C, 3 * CD], FP16, kind="ExternalInput").ap()
    wp = nc.dram_tensor("wp", [CD, C], FP16, kind="ExternalInput").ap()
    y = nc.dram_tensor("y", [T, C], FP16, kind="ExternalOutput").ap()

    with tile.TileContext(nc) as tc:
        with (
            tc.tile_pool(name="consts", bufs=1) as consts,
            tc.tile_pool(name="xt", bufs=2 * CB) as xt_pool,
            tc.tile_pool(name="ew", bufs=8) as ew_pool,
            tc.tile_pool(name="ewd", bufs=6) as ewd_pool,
            tc.tile_pool(name="small", bufs=4) as small_pool,
            tc.tile_pool(name="ysb", bufs=4) as ysb_pool,
            tc.tile_pool(name="psb", bufs=2, space="PSUM") as ps_big,
            tc.tile_pool(name="psm", bufs=4, space="PSUM") as ps_med,
        ):
            # ---- resident tensors ----
            w_qk_t = [
                consts.tile([128, 2 * CD], FP16, name=f"wqk{cb}")
                for cb in range(CB)
            ]
            w_v_t = [
                consts.tile([128, CD], FP16, name=f"wv{cb}")
                for cb in range(CB)
            ]
            wp_sb = consts.tile([128, CD // 128, C], FP16)
            qT = consts.tile([128, NPAIR, T], FP16)
            kT = consts.tile([128, NPAIR, T], FP16)
            # v with 64 replicated ones-columns per head: the AV matmul
            # (M=128) then emits data on one partition half and the
            # softmax denominator replicated across the other half.
            # Even heads (h2=0): [ones | v] -> av rows 0-63 = sums,
            # 64-127 = data.  Odd heads: [v | ones] -> data, sums.
            vS = consts.tile([128, NTB, NPAIR, 2, 2, D], FP16)
            attT = consts.tile([128, NPAIR, T], FP16)

            # PE warmup probe: a matmul with no DMA dependencies, to
            # measure how early the PE can start at all.
            probe_src = consts.tile([128, 128], FP16)
            nc.gpsimd.memset(probe_src, 1.0)
            probe_ps = ps_med.tile([128, 512], FP32, tag="med")
            nc.tensor.matmul(
                probe_ps[:, 0:128], probe_src, probe_src,
                start=True, stop=True,
            )

            # ---- initial DMAs, in first-consumer order ----
            def issue_xt(t):
                xts = []
                for cb in range(CB):
                    xt_t = xt_pool.tile([128, TC], FP16, tag="xt")
                    nc.sync.dma_start(
                        xt_t, xT[128 * cb : 128 * (cb + 1), TC * t : TC * (t + 1)]
                    )
                    xts.append(xt_t)
                return xts

            xts_cur = issue_xt(0)
            w_r = wqkv.rearrange("(po pi) f -> pi po f", pi=128)
            for cb in range(CB):       # q+k columns, one tile per cb
                nc.sync.dma_start(w_qk_t[cb], w_r[:, cb, 0 : 2 * CD])
            for cb in range(CB):       # v columns
                nc.sync.dma_start(w_v_t[cb], w_r[:, cb, 2 * CD : 3 * CD])
            wp_r = wp.rearrange("(po pi) f -> pi po f", pi=128)
            for po in range(CD // 128):
                nc.sync.dma_start(wp_sb[:, po, :], wp_r[:, po, :])
            nc.vector.memset(vS[:, :, :, 0, 0, :], 1.0)
            nc.vector.memset(vS[:, :, :, 1, 1, :], 1.0)

            # -------- global deferred work (software pipeline) --------
            # leftover: closure emitting the previous unit's final AV
            # group; pending: (p, t, av) softmax tail for that unit.
            # Each unit calls flush() after its first PE burst.
            state = {"leftover": None, "pending": None}

            def flush():
                if state["leftover"] is not None:
                    state["leftover"]()
                    state["leftover"] = None
                if state["pending"] is not None:
                    emit_tail(*state["pending"])
                    state["pending"] = None

            # ========== phase 1: qkv for one token chunk ==========
            def qkv_chunk(t, xts):
                # q^T / k^T: one [128, 2*TC] psum holds both column blocks
                for qk in range(2):       # 0 -> q, 1 -> k
                    p2 = ps_big.tile([128, JQK * TC], FP32, tag="big")
                    for jb in range(JQK):
                        co = CD * qk + 128 * jb
                        for cb in range(CB):
                            nc.tensor.matmul(
                                p2[:, TC * jb : TC * (jb + 1)],
                                w_qk_t[cb][:, co : co + 128],
                                xts[cb],
                                start=(cb == 0),
                                stop=(cb == CB - 1),
                            )
                    dst = qT if qk == 0 else kT
                    dst_ap = dst[:, :, TC * t : TC * (t + 1)]
                    src_ap = p2.rearrange("p (j f) -> p j f", j=JQK)
                    if qk == 0:
                        nc.scalar.copy(out=dst_ap, in_=src_ap)
                        flush()       # previous unit's AV + tail
                    else:
                        with nc.allow_low_precision("k copy fp16"):
                            nc.vector.tensor_copy(out=dst_ap, in_=src_ap)
                # v in natural layout, one 128-token block at a time
                for tb in range(TC // 128):
                    tb_g = t * (TC // 128) + tb
                    pvb = ps_big.tile([128, JQK * TC], FP32, tag="big")
                    pv = pvb[:, 0:CD]
                    for cb in range(CB):
                        nc.tensor.matmul(
                            pv,
                            xts[cb][:, 128 * tb : 128 * (tb + 1)],
                            w_v_t[cb],
                            start=(cb == 0),
                            stop=(cb == CB - 1),
                        )
                    pv4 = pv.rearrange("p (pr h d) -> p pr h d", h=2, d=D)
                    with nc.allow_low_precision("v copy fp16"):
                        nc.vector.tensor_copy(
                            out=vS[:, tb_g, :, 0, 1, :], in_=pv4[:, :, 0, :]
                        )
                        nc.vector.tensor_copy(
                            out=vS[:, tb_g, :, 1, 0, :], in_=pv4[:, :, 1, :]
                        )

            # ================= phase 2: attention =================
            def emit_tail(p, ic, av):
                # Pure DVE+DMA tail, all DVE ops in-order on one engine:
                # stage the av psum tile to SBUF fp32 (frees the bank),
                # reciprocal_approx_fast on the full staged tile (only
                # the replicated-sums half is ever read), cast to fp16,
                # DMA partition-shift of the data half into attT,
                # in-place normalize.
                ics = slice(ICH * ic, ICH * (ic + 1))
                for h2 in range(2):
                    po = 64 * h2          # partition offset of the sums
                    s32 = small_pool.tile([128, ICH], FP32, tag="stage")
                    nc.vector.tensor_copy(out=s32, in_=av[h2])
                    r32 = small_pool.tile([128, ICH], FP32, tag="recip")
                    nc.vector.reciprocal_approx_fast(out=r32, in_=s32)
                    t16 = small_pool.tile([128, ICH], FP16, tag="tmp")
                    with nc.allow_low_precision("softmax raw fp16"):
                        nc.vector.tensor_copy(out=t16, in_=s32)
                    # data rows are on the opposite half from the sums
                    dst = attT[po : po + 64, p, ics]
                    nc.sync.dma_start(dst, t16[64 - po : 128 - po, :])
                    nc.vector.tensor_tensor(
                        dst, dst, r32[po : po + 64, :], ALU.mult
                    )

            def attention_ic(p, t):
                n_off = S * t // 2        # off-diagonal groups (of 2 jb)
                J = S * (t + 1)
                off = S * t
                av = [
                    ps_med.tile([128, ICH], FP32, tag="med", name=f"av{h2}")
                    for h2 in range(2)
                ]
                # shared diagonal ew tiles, one per h2: [128, 4, 512] --
                # column j2, query-cols 0:256 from d0 / 256:512 from d1.
                ewds = [None, None]

                def qk_off(g):
                    jbs = (2 * g, 2 * g + 1)
                    ews = []
                    for h2 in range(2):
                        po = 64 * h2
                        sc = ps_big.tile([128, 2 * ICH], FP32, tag="big")
                        for i_s, jb in enumerate(jbs):
                            nc.tensor.matmul(
                                sc[:, ICH * i_s : ICH * (i_s + 1)],
                                kT[po : po + 64, p, 128 * jb : 128 * (jb + 1)],
                                qT[po : po + 64, p, ICH * t : ICH * (t + 1)],
                                start=True,
                                stop=True,
                                tile_position=(po, 0),
                            )
                        ew = ew_pool.tile([128, 2, ICH], FP16, tag="ew")
                        nc.scalar.activation(
                            ew.rearrange("p a b -> p (a b)"),
                            sc,
                            AF.Exp,
                            scale=softmax_scale,
                        )
                        ews.append(ew)
                    return ("off", jbs, ews)

                def qk_diag(qc2):
                    """Diagonal, one 256-query column: key blocks
                    j2 = 0 .. 2*qc2+1 (the two topmost partially
                    masked)."""
                    nj = 2 * qc2 + 2
                    qs = slice(ICH * t + 256 * qc2, ICH * t + 256 * (qc2 + 1))
                    ecol = slice(256 * qc2, 256 * (qc2 + 1))
                    for h2 in range(2):
                        po = 64 * h2
                        if qc2 == 0:
                            ewds[h2] = ewd_pool.tile(
                                [128, S, ICH], FP16, tag="ewd",
                                name=f"ewd{h2}",
                            )
                        ewd = ewds[h2]
                        scb = ps_big.tile([128, 2 * ICH], FP32, tag="big")
                        scd = scb.rearrange("p (a b) -> p a b", b=256)
                        for j2 in range(nj):
                            nc.tensor.matmul(
                                scd[:, j2],
                                kT[
                                    po : po + 64,
                                    p,
                                    128 * (off + j2) : 128 * (off + j2 + 1),
                                ],
                                qT[po : po + 64, p, qs],
                                start=True,
                                stop=True,
                                tile_position=(po, 0),
                            )
                        nc.scalar.activation(
                            ewd[:, 0:nj, ecol],
                            scd[:, 0:nj],
                            AF.Exp,
                            scale=softmax_scale,
                        )
                        # the 2 topmost key blocks are partially masked:
                        # keep iff col - row >= 128*sbit
                        for sbit in range(2):
                            nc.gpsimd.affine_select(
                                out=ewd[:, 2 * qc2 + sbit, ecol],
                                in_=ewd[:, 2 * qc2 + sbit, ecol],
                                compare_op=ALU.is_ge,
                                fill=0.0,
                                base=-128 * sbit,
                                pattern=[[1, 256]],
                                channel_multiplier=-1,
                            )
                    return ("diag", qc2, None)

                def av_grp(grp):
                    kind, idx, ews = grp
                    if kind == "off":
                        for h2 in range(2):
                            for i_s, jb in enumerate(idx):
                                nc.tensor.matmul(
                                    av[h2],
                                    vS[:, jb, p, h2].rearrange(
                                        "p a b -> p (a b)"
                                    ),
                                    ews[h2][:, i_s],
                                    start=(jb == 0),
                                    stop=False,
                                    skip_group_check=True,
                                )
                    else:
                        # diag AV for one 256-query half: qc2=0 needs
                        # key blocks j2 0..1, qc2=1 needs 0..3.
                        qc2 = idx
                        nj = 2 * qc2 + 2
                        acol = slice(256 * qc2, 256 * (qc2 + 1))
                        for h2 in range(2):
                            for j2 in range(nj):
                                nc.tensor.matmul(
                                    av[h2][:, acol],
                                    vS[:, off + j2, p, h2].rearrange(
                                        "p a b -> p (a b)"
                                    ),
                                    ewds[h2][:, j2, acol],
                                    start=(t == 0 and j2 == 0),
                                    stop=(j2 == nj - 1),
                                    skip_group_check=True,
                                )

                def make(s):
                    if s < n_off:
                        return qk_off(s)
                    return qk_diag(s - n_off)

                # global 1-group software pipeline: QK(s+1) before
                # AV(s); the first QK group is followed by the previous
                # unit's deferred AV + tail, and this unit's final AV
                # and tail are deferred into the next unit.
                prev = make(0)
                flush()
                for s in range(1, n_off + 2):
                    cur = make(s)
                    av_grp(prev)
                    prev = cur
                state["leftover"] = lambda g=prev: av_grp(g)
                state["pending"] = (p, t, av)

            # ================= phase 3: projection =================
            def proj_chunk(t):
                for tb in range(t * (TC // 128), (t + 1) * (TC // 128)):
                    for nck in range(C // 512):
                        py = ps_med.tile([128, 512], FP32, tag="med")
                        for p in range(NPAIR):
                            nc.tensor.matmul(
                                py,
                                attT[:, p, 128 * tb : 128 * (tb + 1)],
                                wp_sb[:, p, 512 * nck : 512 * (nck + 1)],
                                start=(p == 0),
                                stop=(p == NPAIR - 1),
                            )
                        y_sb = ysb_pool.tile([128, 512], FP16, tag="ysb")
                        if nck % 2 == 0:
                            nc.scalar.copy(out=y_sb, in_=py)
                        else:
                            with nc.allow_low_precision("y copy fp16"):
                                nc.vector.tensor_copy(out=y_sb, in_=py)
                        nc.sync.dma_start(
                            y[
                                128 * tb : 128 * (tb + 1),
                                512 * nck : 512 * (nck + 1),
                            ],
                            y_sb,
                        )

            def proj_chunk_end(t):
                # Final chunk: emit all p=0 partial matmuls first (they
                # only need the p=0 tail, flushed one unit earlier) so
                # the PE covers the p=1 tail chain; 8 concurrent psum
                # tiles (2 big tiles hold 2 each + 4 med).
                tiles = []
                items = [
                    (tb, nck)
                    for tb in range(t * (TC // 128), (t + 1) * (TC // 128))
                    for nck in range(C // 512)
                ]
                for _ in range(2):
                    if len(tiles) < len(items):
                        big = ps_big.tile([128, 2 * ICH], FP32, tag="big")
                        tiles.append(big[:, 0:512])
                        if len(tiles) < len(items):
                            tiles.append(big[:, 512:1024])
                while len(tiles) < len(items):
                    pym = ps_med.tile(
                        [128, 512], FP32, tag="med", name="pym"
                    )
                    tiles.append(pym)
                for pi in range(NPAIR):
                    for (tb, nck), py in zip(items, tiles):
                        nc.tensor.matmul(
                            py,
                            attT[:, pi, 128 * tb : 128 * (tb + 1)],
                            wp_sb[:, pi, 512 * nck : 512 * (nck + 1)],
                            start=(pi == 0),
                            stop=(pi == NPAIR - 1),
                            skip_group_check=True,
                        )
                for i, ((tb, nck), py) in enumerate(zip(items, tiles)):
                    y_sb = ysb_pool.tile([128, 512], FP16, tag="ysb")
                    if i % 2 == 0:
                        nc.scalar.copy(out=y_sb, in_=py)
                    else:
                        with nc.allow_low_precision("y copy fp16"):
                            nc.vector.tensor_copy(out=y_sb, in_=py)
                    nc.sync.dma_start(
                        y[
                            128 * tb : 128 * (tb + 1),
                            512 * nck : 512 * (nck + 1),
                        ],
                        y_sb,
                    )

            for t in range(NTC):
                qkv_chunk(t, xts_cur)
                if t + 1 < NTC:
                    xts_cur = issue_xt(t + 1)
                if t > 0:
                    proj_chunk(t - 1)
                for p in range(NPAIR):
                    attention_ic(p, t)
            flush()
            proj_chunk_end(NTC - 1)
    nc.compile()
    return nc


def make_in_maps(x, W_qkv, W_proj, n_cores=N_CORES):
    """Host-side sharding: per-core fp16 inputs."""
    Bv, T, C = x.shape
    groups = n_cores // Bv
    CD = C // groups
    in_maps = []
    xT_b = [np.ascontiguousarray(x[b].T).astype(np.float16) for b in range(Bv)]
    for m in range(n_cores):
        b, g = m // groups, m % groups
        cols = slice(CD * g, CD * (g + 1))
        wqkv_dev = np.ascontiguousarray(
            np.concatenate(
                [W_qkv[:, 0:C][:, cols], W_qkv[:, C : 2 * C][:, cols],
                 W_qkv[:, 2 * C : 3 * C][:, cols]],
                axis=1,
            ).astype(np.float16)
        )
        wp_dev = np.ascontiguousarray(W_proj[cols, :]).astype(np.float16)
        in_maps.append({"xT": xT_b[b], "wqkv": wqkv_dev, "wp": wp_dev})
    return in_maps


_cache = {}


def _get_nc():
    if "nc" not in _cache:
        _cache["nc"] = build_nc()
    return _cache["nc"]


def run(x, W_qkv, W_proj, trace=False):
    """Run on hardware; returns (y_full, BassKernelResults)."""
    from concourse.bass_utils import run_bass_kernel_spmd

    nc = _get_nc()
    in_maps = make_in_maps(x, W_qkv, W_proj)
    res = run_bass_kernel_spmd(
        nc, in_maps, core_ids=list(range(N_CORES)), trace=trace
    )
    Bv, T, C = x.shape
    groups = N_CORES // Bv
    y_full = np.zeros((Bv, T, C), dtype=np.float32)
    for m in range(N_CORES):
        y_full[m // groups] += res.results[m]["y"].astype(np.float32)
    return y_full, res


def kernel(x, W_qkv, W_proj):
    y, _ = run(
        np.asarray(x, dtype=np.float32),
        np.asarray(W_qkv, dtype=np.float32),
        np.asarray(W_proj, dtype=np.float32),
    )
    return y
